# revision 7
# baseline (speedup 1.0000x reference)
"""Trainium2 Bass kernel for nn_GTLayer (sparse_attention problem).

Key structural fact about the reference: H == 1 and the softmax is taken
over the HEAD axis, so softmax(attn, axis=0) on a (1, N, N) tensor is
identically 1.0.  Therefore attn @ v reduces to broadcasting the column
sums of v to every row: the A mask, q and k projections are all dead
code.  The attention output row is a single constant vector

    base = (sum_i h_i) @ vw + N * vb, then @ ow + ob

which we compute exactly on the host.  Folding both BatchNorms (eval
mode -> per-feature affine) and the residuals, the whole layer is

    y = h2 + relu(h2 @ W1 + b1) @ W2 + Cfull      (per-feature constants)

with h2 = h * sP.  The device computes only the non-constant FFN part

    F = tv @ W2,   tv = relu(z + b1) - relu(b1) = max(z + min(b1,0), -relu(b1))

(z = h2 @ W1; the max-identity turns relu+bias+subtract into ONE
tensor_scalar) in fp8 e4m3 with DoubleRow matmuls (2 MACs/cell/cycle).
fp8 error is diluted ~1000x because the output is dominated by the
exactly-computed h2 + Cfull part added on the host (measured rel err
~1e-4 vs the 2e-2 gate).

Scaling: weights carry power-of-2 scales (W1*32, W2*512) chosen so
psum1 = 32*z arrives already at fp8-friendly scale for tv (|32*tv| <=
~120 < 240 = TRN e4m3 max); psum2 = 16384*F is scaled back by a DVE
copy (x 1/16384) to bf16 for the output DMA.

Device pipeline per core (1024 rows, everything transposed [feat, row]):
  mm1:  zp  = (32 W1)^T @ h2T        (PE, fp8 DoubleRow, psum f32)
  DVE:  tv8 = max(zp + b1n, -tc)     (one tensor_scalar, psum -> fp8)
  mm2:  fp  = (512 W2)^T @ tv8       (PE, fp8 DoubleRow, psum f32)
  DVE:  y   = fp * 2^-14 -> bf16     (psum -> sbuf)
  DMA out [128, 512] bf16 tiles; host adds h2 + Cfull in fp64.

Rows (N=8192) are sharded over the 8 cores; weights are replicated.
"""

import numpy as np
from contextlib import ExitStack

import ml_dtypes
import concourse.bass as bass
import concourse.mybir as mybir
import concourse.tile as tile
from concourse import bacc
from concourse.bass_utils import run_bass_kernel_spmd

N = 8192
D = 512
H1 = 1024
NCORES = 8
RPC = N // NCORES  # rows per core
EPS = 1e-5
N_WARMUP = 5
S1 = 32.0    # W1 scale (so psum1 = 32*z)
S3 = 512.0   # W2 scale
S23 = S1 * S3

BF16 = mybir.dt.bfloat16
F32 = mybir.dt.float32
FP8 = mybir.dt.float8e4
NPBF16 = np.dtype(ml_dtypes.bfloat16)
NPFP8 = np.dtype(ml_dtypes.float8_e4m3)

KC = D // 128    # 4 contraction chunks in mm1
NC = H1 // 128   # 8 n chunks (mm1 out / mm2 contraction)
DC = D // 128    # 4 d chunks (mm2 out)
RG = RPC // 512  # 2 row groups (matmul free dim 512)


def build_bass():
    nc = bacc.Bacc(
        "TRN2", target_bir_lowering=False, debug=False, num_devices=NCORES
    )
    h8 = nc.dram_tensor("h8", [D, RPC], FP8, kind="ExternalInput")
    w1 = nc.dram_tensor("w1", [D, H1], FP8, kind="ExternalInput")
    w2 = nc.dram_tensor("w2", [H1, D], FP8, kind="ExternalInput")
    # b1n (cols 0..7) and -tc (cols 8..15) packed: one DMA trigger
    bc = nc.dram_tensor("bc", [128, 2 * NC], F32, kind="ExternalInput")
    y = nc.dram_tensor("y", [D, RPC], BF16, kind="ExternalOutput")

    DRM = mybir.MatmulPerfMode.DoubleRow

    with ExitStack() as ctx:
        tc = ctx.enter_context(tile.TileContext(nc))
        consts = ctx.enter_context(tc.tile_pool(name="consts", bufs=1))
        acts = ctx.enter_context(tc.tile_pool(name="acts", bufs=1))
        zpsum = ctx.enter_context(tc.tile_pool(name="zpsum", bufs=4, space="PSUM"))
        fpsum = ctx.enter_context(tc.tile_pool(name="fpsum", bufs=4, space="PSUM"))
        ypool = ctx.enter_context(tc.tile_pool(name="ypool", bufs=3))

        # --- PE warm-up on a memset tile: no DMA dependency, so the PE's
        # HAM activity window fills right after the preamble and real
        # matmuls run at 2.4 GHz instead of 1.2.
        wa = consts.tile([128, 512], BF16)
        nc.vector.memset(wa[:], 0.0)
        # shares the "fp" tag ring with mm2's psum tiles (same shape/dtype)
        # so it costs no extra PSUM bank
        wp = fpsum.tile([128, 512], F32, tag="fp", name="wp")
        for _ in range(N_WARMUP):
            nc.tensor.matmul(wp[:], wa[:, :128], wa[:], start=True, stop=True)

        # --- streaming inputs, critical-path order, few triggers ----------
        w1sb = consts.tile([128, KC, H1], FP8)
        h2sb = acts.tile([128, KC, RPC], FP8)
        w2sb = consts.tile([128, NC, D], FP8)
        bcsb = consts.tile([128, 2 * NC], F32)
        tvsb = acts.tile([128, NC, RPC], FP8)

        W1r = w1.rearrange("(kc p) n -> p kc n", p=128)
        H8r = h8.rearrange("(kc p) r -> p kc r", p=128)
        W2r = w2.rearrange("(c p) d -> p c d", p=128)
        nc.sync.dma_start(w1sb[:, 0:2, :], W1r[:, 0:2, :])
        nc.sync.dma_start(h2sb[:, 0:2, :], H8r[:, 0:2, :])
        nc.sync.dma_start(bcsb[:], bc[:, :])
        nc.sync.dma_start(w1sb[:, 2:4, :], W1r[:, 2:4, :])
        nc.sync.dma_start(h2sb[:, 2:4, :], H8r[:, 2:4, :])
        nc.sync.dma_start(w2sb[:, 0:4, :], W2r[:, 0:4, :])
        nc.sync.dma_start(w2sb[:, 4:8, :], W2r[:, 4:8, :])

        # --- mm1: zp[rg] accumulates 2 DoubleRow matmuls (256-contraction
        # each); weight chunk reused across both row groups.
        for nci in range(NC):
            ns = slice(nci * 128, (nci + 1) * 128)
            zps = [
                zpsum.tile([128, 512], F32, tag="zp", name=f"zp{nci}_{g}")
                for g in range(RG)
            ]
            for i in range(KC // 2):
                ks = slice(2 * i, 2 * i + 2)
                for rg in range(RG):
                    nc.tensor.matmul(
                        zps[rg][:],
                        w1sb[:, ks, ns],
                        h2sb[:, ks, rg * 512 : (rg + 1) * 512],
                        start=(i == 0),
                        stop=(i == KC // 2 - 1),
                        perf_mode=DRM,
                    )
            # tv = relu(z+b1) - relu(b1) == max(z + min(b1,0), -relu(b1)):
            # one two-op tensor_scalar, psum f32 -> sbuf fp8.
            for rg in range(RG):
                nc.vector.tensor_scalar(
                    tvsb[:, nci, rg * 512 : (rg + 1) * 512],
                    zps[rg][:],
                    bcsb[:, nci : nci + 1],
                    bcsb[:, NC + nci : NC + nci + 1],
                    mybir.AluOpType.add,
                    mybir.AluOpType.max,
                )

        # --- mm2: W2 stationary, output F^T tiles [d-chunk, rows].
        Yr = y.rearrange("(dc p) r -> dc p r", p=128)
        for dc in range(DC):
            ds = slice(dc * 128, (dc + 1) * 128)
            fps = [
                fpsum.tile([128, 512], F32, tag="fp", name=f"fp{dc}_{g}")
                for g in range(RG)
            ]
            for i in range(NC // 2):
                ks = slice(2 * i, 2 * i + 2)
                for rg in range(RG):
                    nc.tensor.matmul(
                        fps[rg][:],
                        w2sb[:, ks, ds],
                        tvsb[:, ks, rg * 512 : (rg + 1) * 512],
                        start=(i == 0),
                        stop=(i == NC // 2 - 1),
                        perf_mode=DRM,
                    )
            for rg in range(RG):
                ysb = ypool.tile([128, 512], BF16, tag="ysb")
                nc.vector.tensor_scalar(
                    ysb[:], fps[rg][:], 1.0 / S23, None, mybir.AluOpType.mult
                )
                nc.sync.dma_start(Yr[dc, :, rg * 512 : (rg + 1) * 512], ysb[:])
    nc.compile()
    return nc


_CACHE = {}


def _get_bass():
    if "nc" not in _CACHE:
        _CACHE["nc"] = build_bass()
    return _CACHE["nc"]


def _host_fold(inputs):
    """Fold attention shortcut + BNs into W1, b1, W2, h2, Cfull (float64)."""
    f = lambda k: inputs[k].astype(np.float64)
    h = f("h")
    a1 = f("bn1_g") / np.sqrt(f("bn1_v") + EPS)
    c1 = f("bn1_b") - f("bn1_m") * a1
    a2 = f("bn2_g") / np.sqrt(f("bn2_v") + EPS)
    c2 = f("bn2_b") - f("bn2_m") * a2

    hs = h.sum(axis=0)
    s = hs @ f("vw") + N * f("vb")          # column sums of v
    base = s @ f("ow") + f("ob")            # constant attention-out row
    d1 = base * a1 + c1                     # constant row of bn1(x)
    sP = a1 * a2

    W1 = (1.0 / a2)[:, None] * f("f1w")
    b1 = d1 @ f("f1w") + f("f1b")
    W2 = f("f2w") * a2[None, :]
    C = (d1 + f("f2b")) * a2 + c2

    h2 = h * sP[None, :]

    b1p = (S1 * b1).astype(np.float32)
    b1n = np.minimum(b1p, 0.0)
    mtc = -np.maximum(b1p, 0.0)
    # device computes tv with the exact f32 constants above; fold the
    # same f32 tc into the constant so host+device agree bit-for-bit
    Cfull = C + (np.maximum(b1p, 0.0).astype(np.float64) / S1) @ W2

    pack = lambda v: np.ascontiguousarray(v.reshape(NC, 128).T)
    return {
        "w1": np.ascontiguousarray((W1 * S1).astype(NPFP8)),
        "w2": np.ascontiguousarray((W2 * S3).astype(NPFP8)),
        "bc": np.ascontiguousarray(
            np.concatenate([pack(b1n), pack(mtc)], axis=1).astype(np.float32)
        ),
        "h2": h2,
        "hC": (h2 + Cfull[None, :]).astype(np.float32),
    }


def make_in_maps(inputs):
    hf = _host_fold(inputs)
    in_maps = []
    for c in range(NCORES):
        r0 = c * RPC
        in_maps.append(
            {
                "h8": np.ascontiguousarray(hf["h2"][r0 : r0 + RPC].T).astype(NPFP8),
                "w1": hf["w1"],
                "w2": hf["w2"],
                "bc": hf["bc"],
            }
        )
    return in_maps, hf["hC"]


def kernel(**inputs):
    nc = _get_bass()
    in_maps, hC = make_in_maps(inputs)
    res = run_bass_kernel_spmd(nc, in_maps, core_ids=list(range(NCORES)))
    out = np.empty((N, D), np.float32)
    for c in range(NCORES):
        r0 = c * RPC
        out[r0 : r0 + RPC] = res.results[c]["y"].astype(np.float32).T
    out += hC
    return out


# revision 8
# speedup vs baseline: 1.0453x; 1.0453x over previous
"""Trainium2 Bass kernel for nn_GTLayer (sparse_attention problem).

Key structural fact about the reference: H == 1 and the softmax is taken
over the HEAD axis, so softmax(attn, axis=0) on a (1, N, N) tensor is
identically 1.0.  Therefore attn @ v reduces to broadcasting the column
sums of v to every row: the A mask, q and k projections are all dead
code.  The attention output row is a single constant vector

    base = (sum_i h_i) @ vw + N * vb, then @ ow + ob

which we compute exactly on the host.  Folding both BatchNorms (eval
mode -> per-feature affine) and the residuals, the whole layer is

    y = h2 + relu(h2 @ W1 + b1) @ W2 + Cfull      (per-feature constants)

with h2 = h * sP.  The device computes only the non-constant FFN part

    F = tv @ W2,   tv = relu(z + b1) - relu(b1)

in fp8 e4m3 with DoubleRow matmuls (2 MACs/cell/cycle, ~216ns per
256-contraction x 512-free matmul).  fp8 error is diluted ~1000x
because the output is dominated by the exactly-computed h2 + Cfull part
added on the host (measured rel err ~1e-4 vs the 2e-2 gate).

tv trick: the hidden units are PERMUTED on the host so b1<=0 units come
first.  Then per 128-chunk:
    b1 <= 0 chunk:  tv = relu(z + b1)        -> one ScalarE activation
    b1 >  0 chunk:  tv = max(z, -b1)         -> one VectorE tensor_scalar
    mixed boundary: tv = max(z+min(b1,0), -relu(b1)) -> two-op tensor_scalar
splitting the psum->fp8 conversion work across both engines (the
permutation commutes through the FFN since W2 rows are permuted too).

Scaling: weights carry power-of-2 scales (W1*32, W2*512) chosen so
psum1 = 32*z arrives already at fp8-friendly scale for tv (|32*tv| <=
~120 < 240 = TRN e4m3 max); psum2 = 16384*F is scaled back to bf16 by
the output copy (ScalarE Copy / VectorE mult, alternating).

Rows (N=8192) are sharded over the 8 cores; weights are replicated.
"""

import numpy as np
from contextlib import ExitStack

import ml_dtypes
import concourse.bass as bass
import concourse.mybir as mybir
import concourse.tile as tile
from concourse import bacc
from concourse.bass_utils import run_bass_kernel_spmd

N = 8192
D = 512
H1 = 1024
NCORES = 8
RPC = N // NCORES  # rows per core
EPS = 1e-5
N_WARMUP = 4
S1 = 32.0    # W1 scale (so psum1 = 32*z)
S3 = 512.0   # W2 scale
S23 = S1 * S3

BF16 = mybir.dt.bfloat16
F32 = mybir.dt.float32
FP8 = mybir.dt.float8e4
NPBF16 = np.dtype(ml_dtypes.bfloat16)
NPFP8 = np.dtype(ml_dtypes.float8_e4m3)

KC = D // 128    # 4 contraction chunks in mm1
NC = H1 // 128   # 8 n chunks (mm1 out / mm2 contraction)
DC = D // 128    # 4 d chunks (mm2 out)
RG = RPC // 512  # 2 row groups (matmul free dim 512)


def build_bass(cb):
    """cb = boundary chunk index: chunks < cb have b1<=0, > cb have b1>0."""
    nc = bacc.Bacc(
        "TRN2", target_bir_lowering=False, debug=False, num_devices=NCORES
    )
    h8 = nc.dram_tensor("h8", [D, RPC], FP8, kind="ExternalInput")
    w1 = nc.dram_tensor("w1", [D, H1], FP8, kind="ExternalInput")
    w2 = nc.dram_tensor("w2", [H1, D], FP8, kind="ExternalInput")
    # min(b1,0) (cols 0..7) and -relu(b1) (cols 8..15) packed
    bc = nc.dram_tensor("bc", [128, 2 * NC], F32, kind="ExternalInput")
    y = nc.dram_tensor("y", [D, RPC], BF16, kind="ExternalOutput")

    DRM = mybir.MatmulPerfMode.DoubleRow

    with ExitStack() as ctx:
        tc = ctx.enter_context(tile.TileContext(nc))
        consts = ctx.enter_context(tc.tile_pool(name="consts", bufs=1))
        acts = ctx.enter_context(tc.tile_pool(name="acts", bufs=1))
        zpsum = ctx.enter_context(tc.tile_pool(name="zpsum", bufs=4, space="PSUM"))
        fpsum = ctx.enter_context(tc.tile_pool(name="fpsum", bufs=4, space="PSUM"))
        ypool = ctx.enter_context(tc.tile_pool(name="ypool", bufs=4))

        # --- PE warm-up on a memset tile: no DMA dependency, so the PE's
        # HAM activity window fills right after the preamble and real
        # matmuls run at 2.4 GHz instead of 1.2.  Shares the "fp" psum
        # ring so it costs no extra PSUM bank.
        wa = consts.tile([128, 512], BF16)
        nc.vector.memset(wa[:], 0.0)
        wp = fpsum.tile([128, 512], F32, tag="fp", name="wp")
        for _ in range(N_WARMUP):
            nc.tensor.matmul(wp[:], wa[:, :128], wa[:], start=True, stop=True)

        # --- streaming inputs, critical-path order, few triggers ----------
        w1sb = consts.tile([128, KC, H1], FP8)
        h2sb = acts.tile([128, KC, RPC], FP8)
        w2sb = consts.tile([128, NC, D], FP8)
        bcsb = consts.tile([128, 2 * NC], F32)
        tvsb = acts.tile([128, NC, RPC], FP8)

        W1r = w1.rearrange("(kc p) n -> p kc n", p=128)
        H8r = h8.rearrange("(kc p) r -> p kc r", p=128)
        W2r = w2.rearrange("(c p) d -> p c d", p=128)
        nc.sync.dma_start(w1sb[:, 0:2, :], W1r[:, 0:2, :])
        nc.sync.dma_start(h2sb[:, 0:2, :], H8r[:, 0:2, :])
        nc.sync.dma_start(w1sb[:, 2:4, :], W1r[:, 2:4, :])
        nc.sync.dma_start(h2sb[:, 2:4, :], H8r[:, 2:4, :])
        nc.sync.dma_start(w2sb[:, 0:4, :], W2r[:, 0:4, :])
        nc.sync.dma_start(w2sb[:, 4:8, :], W2r[:, 4:8, :])
        # tiny; issued from the otherwise-idle scalar queue so it doesn't
        # delay the critical sync-queue triggers above
        nc.scalar.dma_start(bcsb[:], bc[:, :])

        # --- mm1: zp[rg] accumulates 2 DoubleRow matmuls (256-contraction
        # each); weight chunk reused across both row groups.
        for nci in range(NC):
            ns = slice(nci * 128, (nci + 1) * 128)
            zps = [
                zpsum.tile([128, 512], F32, tag="zp", name=f"zp{nci}_{g}")
                for g in range(RG)
            ]
            for i in range(KC // 2):
                ks = slice(2 * i, 2 * i + 2)
                for rg in range(RG):
                    nc.tensor.matmul(
                        zps[rg][:],
                        w1sb[:, ks, ns],
                        h2sb[:, ks, rg * 512 : (rg + 1) * 512],
                        start=(i == 0),
                        stop=(i == KC // 2 - 1),
                        perf_mode=DRM,
                    )
            for rg in range(RG):
                dst = tvsb[:, nci, rg * 512 : (rg + 1) * 512]
                if nci < cb:
                    nc.scalar.activation(
                        dst,
                        zps[rg][:],
                        mybir.ActivationFunctionType.Relu,
                        bias=bcsb[:, nci : nci + 1],
                        scale=1.0,
                    )
                elif nci > cb:
                    nc.vector.tensor_scalar(
                        dst,
                        zps[rg][:],
                        bcsb[:, NC + nci : NC + nci + 1],
                        None,
                        mybir.AluOpType.max,
                    )
                else:
                    nc.vector.tensor_scalar(
                        dst,
                        zps[rg][:],
                        bcsb[:, nci : nci + 1],
                        bcsb[:, NC + nci : NC + nci + 1],
                        mybir.AluOpType.add,
                        mybir.AluOpType.max,
                    )

        # --- mm2: W2 stationary, output F^T tiles [d-chunk, rows].
        Yr = y.rearrange("(dc p) r -> dc p r", p=128)
        for dc in range(DC):
            ds = slice(dc * 128, (dc + 1) * 128)
            fps = [
                fpsum.tile([128, 512], F32, tag="fp", name=f"fp{dc}_{g}")
                for g in range(RG)
            ]
            for i in range(NC // 2):
                ks = slice(2 * i, 2 * i + 2)
                for rg in range(RG):
                    nc.tensor.matmul(
                        fps[rg][:],
                        w2sb[:, ks, ds],
                        tvsb[:, ks, rg * 512 : (rg + 1) * 512],
                        start=(i == 0),
                        stop=(i == NC // 2 - 1),
                        perf_mode=DRM,
                    )
            for rg in range(RG):
                ysb = ypool.tile([128, 512], BF16, tag="ysb", name=f"y{dc}_{rg}")
                if rg == 0:
                    nc.scalar.activation(
                        ysb[:],
                        fps[rg][:],
                        mybir.ActivationFunctionType.Copy,
                        bias=0.0,
                        scale=1.0 / S23,
                    )
                else:
                    nc.vector.tensor_scalar(
                        ysb[:], fps[rg][:], 1.0 / S23, None, mybir.AluOpType.mult
                    )
                nc.sync.dma_start(Yr[dc, :, rg * 512 : (rg + 1) * 512], ysb[:])
    nc.compile()
    return nc


_CACHE = {}


def _get_bass(cb):
    if cb not in _CACHE:
        _CACHE[cb] = build_bass(cb)
    return _CACHE[cb]


def _host_fold(inputs):
    """Fold attention shortcut + BNs into W1, b1, W2, h2, Cfull (float64)."""
    f = lambda k: inputs[k].astype(np.float64)
    h = f("h")
    a1 = f("bn1_g") / np.sqrt(f("bn1_v") + EPS)
    c1 = f("bn1_b") - f("bn1_m") * a1
    a2 = f("bn2_g") / np.sqrt(f("bn2_v") + EPS)
    c2 = f("bn2_b") - f("bn2_m") * a2

    hs = h.sum(axis=0)
    s = hs @ f("vw") + N * f("vb")          # column sums of v
    base = s @ f("ow") + f("ob")            # constant attention-out row
    d1 = base * a1 + c1                     # constant row of bn1(x)
    sP = a1 * a2

    W1 = (1.0 / a2)[:, None] * f("f1w")
    b1 = d1 @ f("f1w") + f("f1b")
    W2 = f("f2w") * a2[None, :]
    C = (d1 + f("f2b")) * a2 + c2

    h2 = h * sP[None, :]

    # permute hidden units: b1<=0 first, so per-128-chunk the tv op is a
    # single-engine instruction (see module docstring)
    order = np.argsort(b1 > 0, kind="stable")
    W1 = W1[:, order]
    W2 = W2[order, :]
    b1 = b1[order]
    nneg = int((b1 <= 0).sum())
    cb = nneg // 128

    b1p = (S1 * b1).astype(np.float32)
    b1n = np.minimum(b1p, 0.0)
    mtc = -np.maximum(b1p, 0.0)
    # device computes tv with the exact f32 constants above; fold the
    # same f32 tc into the constant so host+device agree bit-for-bit
    Cfull = C + (np.maximum(b1p, 0.0).astype(np.float64) / S1) @ W2

    pack = lambda v: np.ascontiguousarray(v.reshape(NC, 128).T)
    return {
        "cb": cb,
        "w1": np.ascontiguousarray((W1 * S1).astype(NPFP8)),
        "w2": np.ascontiguousarray((W2 * S3).astype(NPFP8)),
        "bc": np.ascontiguousarray(
            np.concatenate([pack(b1n), pack(mtc)], axis=1).astype(np.float32)
        ),
        "h2": h2,
        "hC": (h2 + Cfull[None, :]).astype(np.float32),
    }


def make_in_maps(inputs):
    hf = _host_fold(inputs)
    in_maps = []
    for c in range(NCORES):
        r0 = c * RPC
        in_maps.append(
            {
                "h8": np.ascontiguousarray(hf["h2"][r0 : r0 + RPC].T).astype(NPFP8),
                "w1": hf["w1"],
                "w2": hf["w2"],
                "bc": hf["bc"],
            }
        )
    return in_maps, hf


def kernel(**inputs):
    in_maps, hf = make_in_maps(inputs)
    nc = _get_bass(hf["cb"])
    res = run_bass_kernel_spmd(nc, in_maps, core_ids=list(range(NCORES)))
    out = np.empty((N, D), np.float32)
    for c in range(NCORES):
        r0 = c * RPC
        out[r0 : r0 + RPC] = res.results[c]["y"].astype(np.float32).T
    out += hf["hC"]
    return out
